# revision 2
# baseline (speedup 1.0000x reference)
"""Sparse-attention Trainium2 kernel (nn_AttentionLayer, B=16 S=2048 D=128).

reference semantics:
    A = Q @ T^T                     # [B,S,S]
    A = where(A > 0.3, A, 0)
    A += where(strictly_upper, -2^32, 0)
    y = softmax(A / sqrt(D)) @ V

Sharding: data-parallel over batch, 2 batches per core on 8 NeuronCores.

Per-core algorithm (per batch):
  - Scores computed transposed, S^T[k,q], 2 ktiles per [128,1024] PSUM
    tile. Straddling-diagonal k-tiles skip dead query columns and are
    left-packed in their PSUM bank so exp spans merge.
  - num = max(exp(S^T*scale),1): ScalarE exp (fp32 PSUM -> bf16 SBUF),
    VectorE tensor_scalar_max (4x mode). Causal mask of each diagonal
    128x128 block is a VectorE multiply with a 0/1 triangle.
  - PV + denominator fused per (ktile, q-subtile): lhsT = num chunk,
    rhs = [V | ones] [128k,129], PSUM-accumulated; obanks packed two
    per PSUM bank.
  - out = PV/den via a VectorE PSUM->SBUF copy + GpSimd normalize_recip.

v2 schedule (vs v1 baseline at 78.4us):
  - All tensors loaded/cast/transposed in qb-aligned PIECES (A=tiles
    0:4, B=4:8, H=8:16) with one tensor per piece so the Tile
    framework's coarse-grained deps never serialize the pipeline.
    QK group g's rhs always lives inside one piece (q spans never
    cross a 4-tile boundary), lhsT/V rhs are single tiles.
  - Load triggers for batch 0 go on the SCALAR queue (ScalarE is idle
    until the first exp ~10us); xbar transposes + batch-1 loads +
    output stores go on the SYNC queue, interleaved so the first QK
    group can start as soon as piece A of q/t is transposed.
  - 8 warm-up matmuls (not 22) ramp the PE p-state during DMA prep.
"""

from collections import deque
from contextlib import ExitStack

import numpy as np

import concourse.bass as bass
import concourse.mybir as mybir
import concourse.tile as tile
from concourse import bacc

B, S, D = 16, 2048, 128
N_CORES = 8
B_LOC = B // N_CORES
QB = 512
N_QB = S // QB
N_ST = S // 128
SCALE = float(1.0 / np.sqrt(D))

F32 = mybir.dt.float32
BF16 = mybir.dt.bfloat16
Alu = mybir.AluOpType

# pieces: name -> (tile_lo, n_tiles)
PIECES = (("A", 0, 4), ("B", 4, 4), ("H", 8, 8))


def piece_of_tile(t):
    """Index into PIECES for the piece containing tile t."""
    return 0 if t < 4 else (1 if t < 8 else 2)


def build_attention_core():
    nc = bacc.Bacc("TRN2", target_bir_lowering=False, debug=False,
                   num_devices=N_CORES)
    q_ext = nc.dram_tensor("Q", [B_LOC, S, D], F32, kind="ExternalInput").ap()
    t_ext = nc.dram_tensor("T", [B_LOC, S, D], F32, kind="ExternalInput").ap()
    v_ext = nc.dram_tensor("V", [B_LOC, S, D], F32, kind="ExternalInput").ap()
    o_ext = nc.dram_tensor("out", [B_LOC, S, D], F32, kind="ExternalOutput").ap()

    with tile.TileContext(nc) as tc, ExitStack() as ctx:
        const_pool = ctx.enter_context(tc.tile_pool(name="const", bufs=1))
        nat_pool = ctx.enter_context(tc.tile_pool(name="nat", bufs=1))
        stage_pool = ctx.enter_context(tc.tile_pool(name="stage", bufs=1))
        tpd_pool = ctx.enter_context(tc.tile_pool(name="tpd", bufs=1))
        vb_pool = ctx.enter_context(tc.tile_pool(name="vb", bufs=1))
        num_pool = ctx.enter_context(tc.tile_pool(name="num", bufs=6))
        fin_pool = ctx.enter_context(tc.tile_pool(name="fin", bufs=3))
        rec_pool = ctx.enter_context(tc.tile_pool(name="rec", bufs=4))
        qk_psum = ctx.enter_context(tc.tile_pool(name="qk_ps", bufs=2, space="PSUM"))
        ob_psum = ctx.enter_context(tc.tile_pool(name="ob_ps", bufs=4, space="PSUM"))

        # ---- constants (gpsimd) ----
        junk = const_pool.tile([128, 512], BF16, name="junk")
        nc.gpsimd.memset(junk[:], 0.25)
        # tri01[p, n] = 0 if p > n else 1 (in-tile causal keep-mask)
        tri01 = const_pool.tile([128, 128], BF16, name="tri01")
        nc.gpsimd.memset(tri01[:], 1.0)
        nc.gpsimd.affine_select(
            out=tri01[:], in_=tri01[:],
            compare_op=Alu.is_ge, fill=0.0,
            base=0, channel_multiplier=-1, pattern=[[1, 128]])

        # ---- per-piece staging tensors ----
        # nat (f32 DMA dst), stg (bf16 cast dst), tp (xbar transpose dst)
        nats = []   # nats[b][which][pi]
        stgs = []
        tps = []    # transposed qt pieces: tps[b]['q'][pi], ['t'][pi]
        vaugs = []  # vaugs[b][pi]: [128, n, 129] bf16 with ones col
        for b in range(B_LOC):
            natb, stgb, tpb, vab = {}, {}, {}, []
            for which in ("q", "t", "v"):
                natb[which] = [
                    nat_pool.tile([128, n, 128], F32, name=f"nat{which}{b}{nm}")
                    for nm, lo, n in PIECES]
            for which in ("q", "t"):
                stgb[which] = [
                    stage_pool.tile([128, n, 128], BF16, name=f"stg{which}{b}{nm}")
                    for nm, lo, n in PIECES]
                tpb[which] = [
                    tpd_pool.tile([128, n, 128], BF16, name=f"tp{which}{b}{nm}")
                    for nm, lo, n in PIECES]
            vab = [vb_pool.tile([128, n, 129], BF16, name=f"vaug{b}{nm}")
                   for nm, lo, n in PIECES]
            nats.append(natb); stgs.append(stgb); tps.append(tpb)
            vaugs.append(vab)

        for b in range(B_LOC):
            for pi, (nm, lo, n) in enumerate(PIECES):
                nc.gpsimd.memset(vaugs[b][pi][:, :, D:D + 1], 1.0)

        def q_tp(b, t):
            """(tensor, local_slot) for transposed q tile t of batch b."""
            pi = piece_of_tile(t)
            return tps[b]["q"][pi], t - PIECES[pi][1]

        def t_tp(b, t):
            pi = piece_of_tile(t)
            return tps[b]["t"][pi], t - PIECES[pi][1]

        def v_tile(b, t):
            pi = piece_of_tile(t)
            return vaugs[b][pi], t - PIECES[pi][1]

        ext_of = {"q": q_ext, "t": t_ext, "v": v_ext}

        def load(b, which, pi, eng):
            nm, lo, n = PIECES[pi]
            eng.dma_start(
                nats[b][which][pi][:],
                ext_of[which][b, 128 * lo:128 * (lo + n), :]
                .rearrange("(t p) d -> p t d", p=128))

        def cast(b, which, pi):
            if which == "v":
                nc.vector.tensor_copy(vaugs[b][pi][:, :, 0:D],
                                      nats[b]["v"][pi][:])
            else:
                nc.vector.tensor_copy(stgs[b][which][pi][:],
                                      nats[b][which][pi][:])

        def transpose(b, which, pi):
            nc.sync.dma_start_transpose(
                tps[b][which][pi][:],
                stgs[b][which][pi][:].rearrange("p t d -> p (t d)"))

        # ---- PE warm-up: ramp the p-state while DMA prep runs ----
        for w in range(8):
            wps = qk_psum.tile([128, 1024], F32, tag="qk", name=f"wps{w}")
            nc.tensor.matmul(wps[:, 0:512], lhsT=junk[:, 0:128], rhs=junk[:])

        # ---- batch-0 loads on the scalar queue (idle until exp starts) ----
        for pi in range(3):
            load(0, "q", pi, nc.scalar)
            load(0, "t", pi, nc.scalar)
            load(0, "v", pi, nc.scalar)

        # casts as data arrives (vector)
        for pi in range(3):
            cast(0, "t", pi)
            cast(0, "q", pi)
            cast(0, "v", pi)

        # transposes on sync, t first (QK lhsT), then q
        transpose(0, "t", 0)
        transpose(0, "q", 0)
        transpose(0, "t", 1)
        transpose(0, "q", 1)
        transpose(0, "t", 2)
        transpose(0, "q", 2)

        # batch-1 loads on sync after batch-0 critical transposes
        for pi in range(3):
            load(1, "q", pi, nc.sync)
            load(1, "t", pi, nc.sync)
            load(1, "v", pi, nc.sync)

        items = []
        for b in range(B_LOC):
            for qb in range(N_QB):
                for g in range((4 * qb + 4) // 2):
                    items.append((b, qb, g))

        def prep_b1(step):
            # spread batch-1 cast/transpose work across batch-0 groups
            if step < 3:
                cast(1, "t", step)
                transpose(1, "t", step)
            elif step < 6:
                pi = step - 3
                cast(1, "q", pi)
                transpose(1, "q", pi)
            elif step < 9:
                cast(1, "v", step - 6)

        prep_at = {10 + i: i for i in range(9)}

        state = {}

        def qk_group(b, qb, g):
            q0 = qb * QB
            s_ps = qk_psum.tile([128, 1024], F32, tag="qk")
            num = num_pool.tile([128, 1024], BF16, tag="num")
            act_spans = []      # merged contiguous spans (left-packed)
            mask_blocks = []    # span starts of diagonal blocks
            last_g = (g == (4 * qb + 4) // 2 - 1)
            for j, c in enumerate((2 * g, 2 * g + 1)):
                i = c - 4 * qb
                lo = 128 * i if i > 0 else 0
                w = QB - lo
                ql = q0 + lo
                t0_ = ql // 128
                nt = (QB - lo) // 128
                # the final (i2,i3) pair packs into one bank: j1 at col 256
                s0 = 256 if (last_g and j == 1) else j * 512
                q_tens, q_lo = q_tp(b, t0_)
                rhs = q_tens[:, q_lo:q_lo + nt, :] \
                    .rearrange("p t q -> p (t q)")
                t_tens, t_lo = t_tp(b, c)
                nc.tensor.matmul(
                    s_ps[:, s0:s0 + w],
                    lhsT=t_tens[:, t_lo, :],
                    rhs=rhs,
                    start=not (last_g and j == 1), stop=True,
                    skip_group_check=(last_g and j == 1),
                )
                if act_spans and act_spans[-1][1] == s0:
                    act_spans[-1] = (act_spans[-1][0], s0 + w)
                else:
                    act_spans.append((s0, s0 + w))
                if i >= 0:
                    mask_blocks.append(s0)
            for lo_, hi_ in act_spans:
                nc.scalar.activation(num[:, lo_:hi_], s_ps[:, lo_:hi_],
                                     mybir.ActivationFunctionType.Exp,
                                     scale=SCALE)
                nc.vector.tensor_scalar_max(num[:, lo_:hi_],
                                            num[:, lo_:hi_], 1.0)
            for ds in mask_blocks:
                nc.vector.tensor_tensor(num[:, ds:ds + 128],
                                        num[:, ds:ds + 128], tri01[:],
                                        op=Alu.mult)
            st = state.setdefault((b, qb), {"ob": None, "num": {}})
            if st["ob"] is None:
                st["ob"] = [ob_psum.tile([128, 2, 256], F32, tag="ob",
                                         name=f"ob_{b}_{qb}_{h}")
                            for h in range(2)]
            st["num"][g] = num

        def pv_group(b, qb, g):
            st = state[(b, qb)]
            num = st["num"].pop(g)
            last_g = (g == (4 * qb + 4) // 2 - 1)
            for j, c in enumerate((2 * g, 2 * g + 1)):
                i = c - 4 * qb
                lo = 128 * i if i > 0 else 0
                s0 = 256 if (last_g and j == 1) else j * 512
                v_tens, v_lo = v_tile(b, c)
                for sub in range(max(i, 0), 4):
                    ob = st["ob"][sub // 2]
                    nc.tensor.matmul(
                        ob[:, sub % 2, 0:129],
                        lhsT=num[:, s0 + sub * 128 - lo:
                                 s0 + (sub + 1) * 128 - lo],
                        rhs=v_tens[:, v_lo, 0:129],
                        start=(c == 0 and sub % 2 == 0),
                        stop=(c == 4 * qb + sub),
                        skip_group_check=True,
                    )

        def finalize(b, qb):
            st = state.pop((b, qb))
            o_tile = fin_pool.tile([128, 4, 128], F32, tag="fin")
            for h in range(2):
                ob_sb = rec_pool.tile([128, 2, 129], F32, tag="rec")
                nc.vector.tensor_copy(ob_sb[:], st["ob"][h][:, :, 0:129])
                for s2 in range(2):
                    nc.gpsimd.normalize_recip(
                        o_tile[:, 2 * h + s2, :],
                        ob_sb[:, s2, 0:128],
                        ob_sb[:, s2, 128:129])
            nc.sync.dma_start(
                o_ext[b, qb * QB:(qb + 1) * QB, :]
                    .rearrange("(s p) d -> p s d", p=128),
                o_tile[:])

        pending = deque()

        def flush_one():
            b, qb, g = pending.popleft()
            pv_group(b, qb, g)
            if g == (4 * qb + 4) // 2 - 1:
                finalize(b, qb)

        n_items = len(items)
        for idx, it in enumerate(items):
            qk_group(*it)
            if idx in prep_at:
                prep_b1(prep_at[idx])
            pending.append(it)
            # drain the pipeline harder near the end so the tail is short
            depth = 2 if idx < n_items - 4 else 1
            while len(pending) > depth:
                flush_one()
        while pending:
            flush_one()

    nc.compile()
    return nc


_NC_CACHE = None


def _get_nc():
    global _NC_CACHE
    if _NC_CACHE is None:
        _NC_CACHE = build_attention_core()
    return _NC_CACHE


def kernel(Q: np.ndarray, T: np.ndarray, V: np.ndarray) -> np.ndarray:
    """Full-input entry point: shard over batch, run 8-core SPMD, gather."""
    from concourse.bass_utils import run_bass_kernel_spmd

    Q = np.ascontiguousarray(np.asarray(Q, dtype=np.float32))
    T = np.ascontiguousarray(np.asarray(T, dtype=np.float32))
    V = np.ascontiguousarray(np.asarray(V, dtype=np.float32))
    assert Q.shape == (B, S, D), Q.shape

    nc = _get_nc()
    in_maps = [
        {
            "Q": Q[i * B_LOC:(i + 1) * B_LOC],
            "T": T[i * B_LOC:(i + 1) * B_LOC],
            "V": V[i * B_LOC:(i + 1) * B_LOC],
        }
        for i in range(N_CORES)
    ]
    res = run_bass_kernel_spmd(nc, in_maps, core_ids=list(range(N_CORES)))
    return np.concatenate([res.results[i]["out"] for i in range(N_CORES)], axis=0)


# revision 7
# speedup vs baseline: 1.0464x; 1.0464x over previous
"""Sparse-attention Trainium2 kernel (nn_AttentionLayer, B=16 S=2048 D=128).

reference semantics:
    A = Q @ T^T                     # [B,S,S]
    A = where(A > 0.3, A, 0)
    A += where(strictly_upper, -2^32, 0)
    y = softmax(A / sqrt(D)) @ V

Sharding: data-parallel over batch, 2 batches per core on 8 NeuronCores.

Per-core algorithm (per batch):
  - Scores computed transposed, S^T[k,q], 2 ktiles per [128,1024] PSUM
    tile. Straddling-diagonal k-tiles skip dead query columns and are
    left-packed in their PSUM bank so exp spans merge.
  - num = max(exp(S^T*scale),1): ScalarE exp (fp32 PSUM -> bf16 SBUF),
    VectorE tensor_scalar_max (4x mode). Causal mask of each diagonal
    128x128 block is a VectorE multiply with a 0/1 triangle.
  - PV + denominator fused per (ktile, q-subtile): lhsT = num chunk,
    rhs = [V | ones] [128k,129], PSUM-accumulated; obanks packed two
    per PSUM bank.
  - out = PV/den via a VectorE PSUM->SBUF copy + GpSimd normalize_recip.

v2 schedule (vs v1 baseline at 78.4us):
  - All tensors loaded/cast/transposed in qb-aligned PIECES (A=tiles
    0:4, B=4:8, H=8:16) with one tensor per piece so the Tile
    framework's coarse-grained deps never serialize the pipeline.
    QK group g's rhs always lives inside one piece (q spans never
    cross a 4-tile boundary), lhsT/V rhs are single tiles.
  - Load triggers for batch 0 go on the SCALAR queue (ScalarE is idle
    until the first exp ~10us); xbar transposes + batch-1 loads +
    output stores go on the SYNC queue, interleaved so the first QK
    group can start as soon as piece A of q/t is transposed.
  - 8 warm-up matmuls (not 22) ramp the PE p-state during DMA prep.
"""

from collections import deque
from contextlib import ExitStack

import numpy as np

import concourse.bass as bass
import concourse.mybir as mybir
import concourse.tile as tile
from concourse import bacc

B, S, D = 16, 2048, 128
N_CORES = 8
B_LOC = B // N_CORES
QB = 512
N_QB = S // QB
N_ST = S // 128
SCALE = float(1.0 / np.sqrt(D))

F32 = mybir.dt.float32
BF16 = mybir.dt.bfloat16
Alu = mybir.AluOpType

# pieces: name -> (tile_lo, n_tiles). QK rhs spans never cross tile 4.
PIECES = (("A", 0, 4), ("R", 4, 12))


def piece_of_tile(t):
    """Index into PIECES for the piece containing tile t."""
    return 0 if t < 4 else 1


def build_attention_core():
    nc = bacc.Bacc("TRN2", target_bir_lowering=False, debug=False,
                   num_devices=N_CORES)
    q_ext = nc.dram_tensor("Q", [B_LOC, S, D], F32, kind="ExternalInput").ap()
    t_ext = nc.dram_tensor("T", [B_LOC, S, D], F32, kind="ExternalInput").ap()
    v_ext = nc.dram_tensor("V", [B_LOC, S, D], F32, kind="ExternalInput").ap()
    o_ext = nc.dram_tensor("out", [B_LOC, S, D], F32, kind="ExternalOutput").ap()

    with tile.TileContext(nc) as tc, ExitStack() as ctx:
        const_pool = ctx.enter_context(tc.tile_pool(name="const", bufs=1))
        nat_pool = ctx.enter_context(tc.tile_pool(name="nat", bufs=1))
        stage_pool = ctx.enter_context(tc.tile_pool(name="stage", bufs=1))
        tpd_pool = ctx.enter_context(tc.tile_pool(name="tpd", bufs=1))
        vb_pool = ctx.enter_context(tc.tile_pool(name="vb", bufs=1))
        num_pool = ctx.enter_context(tc.tile_pool(name="num", bufs=6))
        fin_pool = ctx.enter_context(tc.tile_pool(name="fin", bufs=3))
        rec_pool = ctx.enter_context(tc.tile_pool(name="rec", bufs=4))
        qk_psum = ctx.enter_context(tc.tile_pool(name="qk_ps", bufs=2, space="PSUM"))
        ob_psum = ctx.enter_context(tc.tile_pool(name="ob_ps", bufs=4, space="PSUM"))

        # ---- constants (gpsimd) ----
        junk = const_pool.tile([128, 512], BF16, name="junk")
        nc.gpsimd.memset(junk[:], 0.25)
        # tri01[p, n] = 0 if p > n else 1 (in-tile causal keep-mask)
        tri01 = const_pool.tile([128, 128], BF16, name="tri01")
        nc.gpsimd.memset(tri01[:], 1.0)
        nc.gpsimd.affine_select(
            out=tri01[:], in_=tri01[:],
            compare_op=Alu.is_ge, fill=0.0,
            base=0, channel_multiplier=-1, pattern=[[1, 128]])

        # ---- per-piece staging tensors ----
        # nat (f32 DMA dst), stg (bf16 cast dst), tp (xbar transpose dst)
        nats = []   # nats[b][which][pi]
        stgs = []
        tps = []    # transposed qt pieces: tps[b]['q'][pi], ['t'][pi]
        vaugs = []  # vaugs[b][pi]: [128, n, 129] bf16 with ones col
        for b in range(B_LOC):
            natb, stgb, tpb, vab = {}, {}, {}, []
            for which in ("q", "t", "v"):
                natb[which] = [
                    nat_pool.tile([128, n, 128], F32, name=f"nat{which}{b}{nm}")
                    for nm, lo, n in PIECES]
            for which in ("q", "t"):
                stgb[which] = [
                    stage_pool.tile([128, n, 128], BF16, name=f"stg{which}{b}{nm}")
                    for nm, lo, n in PIECES]
                tpb[which] = [
                    tpd_pool.tile([128, n, 128], BF16, name=f"tp{which}{b}{nm}")
                    for nm, lo, n in PIECES]
            vab = [vb_pool.tile([128, n, 129], BF16, name=f"vaug{b}{nm}")
                   for nm, lo, n in PIECES]
            nats.append(natb); stgs.append(stgb); tps.append(tpb)
            vaugs.append(vab)

        for b in range(B_LOC):
            for pi, (nm, lo, n) in enumerate(PIECES):
                nc.gpsimd.memset(vaugs[b][pi][:, :, D:D + 1], 1.0)

        # batch-1 loads land in whole-tensor nat buffers (one DMA each)
        nat1 = {which: nat_pool.tile([128, 16, 128], F32, name=f"nat1{which}")
                for which in ("q", "t", "v")}

        def q_tp(b, t):
            """(tensor, local_slot) for transposed q tile t of batch b."""
            pi = piece_of_tile(t)
            return tps[b]["q"][pi], t - PIECES[pi][1]

        def t_tp(b, t):
            pi = piece_of_tile(t)
            return tps[b]["t"][pi], t - PIECES[pi][1]

        def v_tile(b, t):
            pi = piece_of_tile(t)
            return vaugs[b][pi], t - PIECES[pi][1]

        ext_of = {"q": q_ext, "t": t_ext, "v": v_ext}

        def load(b, which, pi, eng):
            nm, lo, n = PIECES[pi]
            eng.dma_start(
                nats[b][which][pi][:],
                ext_of[which][b, 128 * lo:128 * (lo + n), :]
                .rearrange("(t p) d -> p t d", p=128))

        def load_whole_b1(which, eng):
            eng.dma_start(
                nat1[which][:],
                ext_of[which][1].rearrange("(t p) d -> p t d", p=128))

        def cast(b, which, pi):
            nm, lo, n = PIECES[pi]
            src = (nats[b][which][pi][:] if b == 0
                   else nat1[which][:, lo:lo + n, :])
            if which == "v":
                nc.vector.tensor_copy(vaugs[b][pi][:, :, 0:D], src)
            else:
                nc.vector.tensor_copy(stgs[b][which][pi][:], src)

        def transpose(b, which, pi):
            nc.sync.dma_start_transpose(
                tps[b][which][pi][:],
                stgs[b][which][pi][:].rearrange("p t d -> p (t d)"))

        # ---- PE warm-up: ramp the p-state while DMA prep runs ----
        for w in range(8):
            wps = qk_psum.tile([128, 1024], F32, tag="qk", name=f"wps{w}")
            nc.tensor.matmul(wps[:, 0:512], lhsT=junk[:, 0:128], rhs=junk[:])

        # ---- batch-0 loads: q/t on sync, v on scalar (only 2 triggers
        # so ScalarE is free for exp from ~9us on) ----
        load(0, "t", 0, nc.sync)
        load(0, "q", 0, nc.sync)
        load(0, "t", 1, nc.sync)
        load(0, "q", 1, nc.sync)
        load(0, "v", 0, nc.scalar)
        load(0, "v", 1, nc.scalar)

        # casts as data arrives (vector)
        cast(0, "t", 0)
        cast(0, "q", 0)
        cast(0, "v", 0)
        cast(0, "t", 1)
        cast(0, "q", 1)
        cast(0, "v", 1)

        # transposes on sync (in-order after the 4 q/t triggers)
        transpose(0, "t", 0)
        transpose(0, "q", 0)
        transpose(0, "t", 1)
        transpose(0, "q", 1)

        # batch-1 loads on sync after batch-0 critical transposes
        load_whole_b1("t", nc.sync)
        load_whole_b1("q", nc.sync)
        load_whole_b1("v", nc.sync)

        items = []
        for b in range(B_LOC):
            for qb in range(N_QB):
                for g in range((4 * qb + 4) // 2):
                    items.append((b, qb, g))

        def fillers(n):
            # junk matmuls to keep the PE busy (p-state ramp / bubbles)
            for _ in range(n):
                wps = qk_psum.tile([128, 1024], F32, tag="qk")
                nc.tensor.matmul(wps[:, 0:512], lhsT=junk[:, 0:128],
                                 rhs=junk[:])

        def prep_b1(step):
            # spread batch-1 cast/transpose work across batch-0 groups
            if step == 0:
                cast(1, "t", 0)
                transpose(1, "t", 0)
            elif step == 1:
                cast(1, "q", 0)
                transpose(1, "q", 0)
            elif step == 2:
                cast(1, "v", 0)
            elif step == 3:
                cast(1, "t", 1)
                transpose(1, "t", 1)
            elif step == 4:
                cast(1, "q", 1)
                transpose(1, "q", 1)
            elif step == 5:
                cast(1, "v", 1)

        prep_at = {10 + 2 * i: i for i in range(6)}

        state = {}

        def qk_group(b, qb, g):
            q0 = qb * QB
            s_ps = qk_psum.tile([128, 1024], F32, tag="qk")
            num = num_pool.tile([128, 1024], BF16, tag="num")
            act_spans = []      # merged contiguous spans (left-packed)
            mask_blocks = []    # span starts of diagonal blocks
            last_g = (g == (4 * qb + 4) // 2 - 1)
            for j, c in enumerate((2 * g, 2 * g + 1)):
                i = c - 4 * qb
                lo = 128 * i if i > 0 else 0
                w = QB - lo
                ql = q0 + lo
                t0_ = ql // 128
                nt = (QB - lo) // 128
                # the final (i2,i3) pair packs into one bank: j1 at col 256
                s0 = 256 if (last_g and j == 1) else j * 512
                q_tens, q_lo = q_tp(b, t0_)
                rhs = q_tens[:, q_lo:q_lo + nt, :] \
                    .rearrange("p t q -> p (t q)")
                t_tens, t_lo = t_tp(b, c)
                nc.tensor.matmul(
                    s_ps[:, s0:s0 + w],
                    lhsT=t_tens[:, t_lo, :],
                    rhs=rhs,
                    start=not (last_g and j == 1), stop=True,
                    skip_group_check=(last_g and j == 1),
                )
                if act_spans and act_spans[-1][1] == s0:
                    act_spans[-1] = (act_spans[-1][0], s0 + w)
                else:
                    act_spans.append((s0, s0 + w))
                if i >= 0:
                    mask_blocks.append(s0)
            for lo_, hi_ in act_spans:
                nc.scalar.activation(num[:, lo_:hi_], s_ps[:, lo_:hi_],
                                     mybir.ActivationFunctionType.Exp,
                                     scale=SCALE)
                nc.vector.tensor_scalar_max(num[:, lo_:hi_],
                                            num[:, lo_:hi_], 1.0)
            for ds in mask_blocks:
                nc.vector.tensor_tensor(num[:, ds:ds + 128],
                                        num[:, ds:ds + 128], tri01[:],
                                        op=Alu.mult)
            st = state.setdefault((b, qb), {"ob": None, "num": {}})
            if st["ob"] is None:
                st["ob"] = [ob_psum.tile([128, 2, 256], F32, tag="ob",
                                         name=f"ob_{b}_{qb}_{h}")
                            for h in range(2)]
            st["num"][g] = num

        def pv_group(b, qb, g):
            st = state[(b, qb)]
            num = st["num"].pop(g)
            last_g = (g == (4 * qb + 4) // 2 - 1)
            for j, c in enumerate((2 * g, 2 * g + 1)):
                i = c - 4 * qb
                lo = 128 * i if i > 0 else 0
                s0 = 256 if (last_g and j == 1) else j * 512
                v_tens, v_lo = v_tile(b, c)
                for sub in range(max(i, 0), 4):
                    ob = st["ob"][sub // 2]
                    nc.tensor.matmul(
                        ob[:, sub % 2, 0:129],
                        lhsT=num[:, s0 + sub * 128 - lo:
                                 s0 + (sub + 1) * 128 - lo],
                        rhs=v_tens[:, v_lo, 0:129],
                        start=(c == 0 and sub % 2 == 0),
                        stop=(c == 4 * qb + sub),
                        skip_group_check=True,
                    )

        def finalize(b, qb):
            st = state.pop((b, qb))
            o_tile = fin_pool.tile([128, 4, 128], F32, tag="fin")
            for h in range(2):
                ob_sb = rec_pool.tile([128, 2, 129], F32, tag="rec")
                nc.vector.tensor_copy(ob_sb[:], st["ob"][h][:, :, 0:129])
                for s2 in range(2):
                    nc.gpsimd.normalize_recip(
                        o_tile[:, 2 * h + s2, :],
                        ob_sb[:, s2, 0:128],
                        ob_sb[:, s2, 128:129])
            nc.sync.dma_start(
                o_ext[b, qb * QB:(qb + 1) * QB, :]
                    .rearrange("(s p) d -> p s d", p=128),
                o_tile[:])

        pending = deque()

        def flush_one():
            b, qb, g = pending.popleft()
            pv_group(b, qb, g)
            if g == (4 * qb + 4) // 2 - 1:
                finalize(b, qb)

        n_items = len(items)
        for idx, it in enumerate(items):
            qk_group(*it)
            if idx == 1:
                # bridge the PE bubble while the R pieces transpose
                fillers(6)
            if idx in prep_at:
                prep_b1(prep_at[idx])
            pending.append(it)
            # drain the pipeline harder near the end so the tail is short
            depth = 2 if idx < n_items - 4 else 1
            while len(pending) > depth:
                flush_one()
        while pending:
            flush_one()

    nc.compile()
    return nc


_NC_CACHE = None


def _get_nc():
    global _NC_CACHE
    if _NC_CACHE is None:
        _NC_CACHE = build_attention_core()
    return _NC_CACHE


def kernel(Q: np.ndarray, T: np.ndarray, V: np.ndarray) -> np.ndarray:
    """Full-input entry point: shard over batch, run 8-core SPMD, gather."""
    from concourse.bass_utils import run_bass_kernel_spmd

    Q = np.ascontiguousarray(np.asarray(Q, dtype=np.float32))
    T = np.ascontiguousarray(np.asarray(T, dtype=np.float32))
    V = np.ascontiguousarray(np.asarray(V, dtype=np.float32))
    assert Q.shape == (B, S, D), Q.shape

    nc = _get_nc()
    in_maps = [
        {
            "Q": Q[i * B_LOC:(i + 1) * B_LOC],
            "T": T[i * B_LOC:(i + 1) * B_LOC],
            "V": V[i * B_LOC:(i + 1) * B_LOC],
        }
        for i in range(N_CORES)
    ]
    res = run_bass_kernel_spmd(nc, in_maps, core_ids=list(range(N_CORES)))
    return np.concatenate([res.results[i]["out"] for i in range(N_CORES)], axis=0)
